# revision 10
# baseline (speedup 1.0000x reference)
"""ChineseCLIPVisionLayer on 8 trn2 NeuronCores.

Sharding: pure data-parallel over batch (B=32 -> 4 per core), zero
collectives. Weights are host-transposed and replicated to every core.

Per-core pipeline (all activations that feed matmuls live in transposed
layout [D, S] so the contraction dim sits on SBUF partitions):
  LN1 (natural) -> PE-transpose -> h^T
  q^T,k^T (transposed out), v (natural out)
  per head: scores = q_h^T.T @ k_h^T -> softmax (no max-sub; scores ~ +-3)
            probs -> PE-transpose -> probs^T
            attn^T = v_h.T @ probs^T   (+v_b via bias: softmax rows sum to 1)
  out_proj -> attn_out^T -> PE-transpose back + residual -> x1 (natural)
  LN2 -> h2^T ; MLP in 2 batch-groups (weights streamed once per group)
  quick-gelu == Gelu_apprx_sigmoid on ACT; fc2 out -> transpose + residual.
Matmuls run in float32r (full PE rate at N>=256; fp32 would be 4x slower).
Pools are stage-scoped (manually closed ExitStacks) to fit SBUF.
"""

from contextlib import ExitStack

import numpy as np

import concourse.bass as bass
import concourse.mybir as mybir
import concourse.tile as tile
from concourse import bacc, bass_utils
from concourse.masks import make_identity

N_CORES = 8
B, S, D = 32, 257, 1024
H, HD = 16, 64
FF = 4096
EPS = 1e-5
SCALE = HD ** -0.5
NB = B // N_CORES  # batch elems per core

F32 = mybir.dt.float32
F32R = mybir.dt.float32r
AF = mybir.ActivationFunctionType
ALU = mybir.AluOpType

# sequence chunks (partition-dim tiling of S=257)
SQ = [(0, 128), (128, 128), (256, 1)]
SE = 258  # free-dim padded length: fp32r matmul needs even moving/dst size
DC = D // 128   # 8 chunks of the model dim
FC = FF // 128  # 32 chunks of the ff dim


def r(ap):
    return ap.bitcast(F32R)


def build():
    nc = bacc.Bacc("TRN2", target_bir_lowering=False, debug=False,
                   num_devices=N_CORES)

    def din(name, shape, dt=F32):
        return nc.dram_tensor(name, shape, dt, kind="ExternalInput").ap()

    x_d = din("x", [NB, S, D])
    qwT_d = din("qwT", [D, D], F32R)
    kwT_d = din("kwT", [D, D], F32R)
    vwT_d = din("vwT", [D, D], F32R)
    owT_d = din("owT", [D, D], F32R)
    f1wT_d = din("f1wT", [D, FF], F32R)
    f2wT_d = din("f2wT", [FF, D], F32R)
    qb_d = din("qb", [D])
    kb_d = din("kb", [D])
    vb_d = din("vb", [D])
    ob_d = din("ob", [D])
    f1b_d = din("f1b", [FF])
    f2b_d = din("f2b", [D])
    g1_d = din("g1", [D])
    b1_d = din("b1", [D])
    g2_d = din("g2", [D])
    b2_d = din("b2", [D])
    out_d = nc.dram_tensor("out", [NB, S, D], F32, kind="ExternalOutput").ap()

    with tile.TileContext(nc) as tc:
        with ExitStack() as es:
            P = lambda name, bufs, **kw: es.enter_context(
                tc.tile_pool(name=name, bufs=bufs, **kw))
            const = P("const", 1)
            biasp = P("bias", 1)
            xio = P("xio", 2)
            stat = P("stat", 8)
            pp = P("pp", 4, space="PSUM")
            pt = P("pt", 2, space="PSUM")
            dramp = P("dram", 1, space="DRAM")

            ident = const.tile([128, 128], F32)
            make_identity(nc, ident)
            epsc = const.tile([128, 1], F32)
            nc.vector.memset(epsc[:], EPS)

            def load_bias(dram, n):
                t = biasp.tile([128, n // 128], F32, name=f"bias_{dram.name}")
                nc.sync.dma_start(t[:], dram.rearrange("(c p) -> p c", p=128))
                return t

            qb_sb = load_bias(qb_d, D)
            kb_sb = load_bias(kb_d, D)
            vb_sb = load_bias(vb_d, D)
            ob_sb = load_bias(ob_d, D)
            f1b_sb = load_bias(f1b_d, FF)
            f2b_sb = load_bias(f2b_d, D)
            g1_sb = load_bias(g1_d, D)
            b1_sb = load_bias(b1_d, D)
            g2_sb = load_bias(g2_d, D)
            b2_sb = load_bias(b2_d, D)

            x1_scr = dramp.tile([NB, S, D], F32)

            def layer_norm(src_tiles, hpool):
                """src_tiles: 3 natural tiles [(pz, D)]; returns normalized
                (x-mu)*rstd tiles (gamma/beta applied at transpose evict)."""
                out_tiles = []
                for j, (o, pz) in enumerate(SQ):
                    xt = src_tiles[j]
                    st = stat.tile([pz, 2, 6], F32, name="st", tag="st")
                    nc.vector.bn_stats(st[:, 0, :], xt[:, 0:512])
                    nc.vector.bn_stats(st[:, 1, :], xt[:, 512:1024])
                    mv = stat.tile([pz, 2], F32, name="mv", tag="mv")
                    nc.vector.bn_aggr(mv[:], st[:])
                    rstd = stat.tile([pz, 1], F32, name="rstd", tag="rstd")
                    nc.scalar.activation(rstd[:], mv[:, 1:2], AF.Sqrt,
                                         bias=epsc[:pz, :])
                    nc.vector.reciprocal(rstd[:], rstd[:])
                    ht = hpool.tile([pz, D], F32, name="hn", tag="hn")
                    nc.vector.tensor_scalar(
                        out=ht[:], in0=xt[:], scalar1=mv[:, 0:1],
                        scalar2=rstd[:], op0=ALU.subtract, op1=ALU.mult)
                    out_tiles.append(ht)
                return out_tiles

            def transpose_to_T(nat_tiles, dst_pool, g_sb, bt_sb, tag):
                """nat tiles [(pz, D)] -> 8 tiles [128, S] of the transpose,
                evicted with per-partition scale g and bias bt."""
                outs = []
                for dc in range(DC):
                    ps = pt.tile([128, SE], F32, name="psT", tag="pt")
                    for j, (o, pz) in enumerate(SQ):
                        nc.tensor.transpose(
                            ps[:, o:o + pz],
                            nat_tiles[j][:, dc * 128:(dc + 1) * 128],
                            ident[:pz, :pz])
                    t = dst_pool.tile([128, SE], F32R, name=f"{tag}", tag=tag)
                    if g_sb is None:
                        nc.scalar.copy(t[:], ps[:])
                    else:
                        nc.scalar.activation(
                            t[:], ps[:], AF.Identity,
                            bias=bt_sb[:, dc:dc + 1], scale=g_sb[:, dc:dc + 1])
                    outs.append(t)
                return outs

            def project_T(wT_dram, rhs_per_b, bias_sb, dst_pool, tag, wpool,
                          func=AF.Identity, odt=F32R):
                """y^T = wT.T @ rhs (+bias) for every batch elem.
                Streams wT in two m-halves of [128, DC, 512] to bound SBUF.
                Returns outs[b][mc] tiles [128, S]."""
                src = wT_dram.rearrange("(kc p) m -> p kc m", p=128)
                outs = [[None] * DC for _ in range(NB)]
                for half in range(2):
                    wt = wpool.tile([128, DC, 512], F32R, name="pw", tag="pw")
                    for kc in range(DC):
                        nc.sync.dma_start(
                            wt[:, kc, :],
                            src[:, kc, half * 512:(half + 1) * 512])
                    for b in range(NB):
                        for ml in range(4):
                            mc = half * 4 + ml
                            ps = pp.tile([128, SE], F32, name="psP", tag="pp")
                            for kc in range(DC):
                                nc.tensor.matmul(
                                    ps[:],
                                    wt[:, kc, ml * 128:(ml + 1) * 128],
                                    rhs_per_b[b][kc][:],
                                    start=(kc == 0), stop=(kc == DC - 1))
                            t = dst_pool.tile([128, SE], odt, name=tag,
                                              tag=tag)
                            nc.scalar.activation(t[:], ps[:], func,
                                                 bias=bias_sb[:, mc:mc + 1])
                            outs[b][mc] = t
                return outs

            # ---------- stage A: load x, LN1, h^T ----------
            esA_HT = ExitStack()
            HTp = esA_HT.enter_context(
                tc.tile_pool(name="HT", bufs=NB * DC, side="right"))
            esA = ExitStack()
            hnat = esA.enter_context(
                tc.tile_pool(name="hnat", bufs=3, side="right"))
            HT = []
            for b in range(NB):
                xts = []
                for j, (o, pz) in enumerate(SQ):
                    xt = xio.tile([pz, D], F32, name="xin", tag="xin")
                    nc.sync.dma_start(xt[:], x_d[b, o:o + pz, :])
                    xts.append(xt)
                hts = layer_norm(xts, hnat)
                HT.append(transpose_to_T(hts, HTp, g1_sb, b1_sb, "HT"))

            # ---------- stage B: QKV ----------
            esA.close()  # hnat dead
            esBC = ExitStack()
            qTp = esBC.enter_context(tc.tile_pool(name="qT", bufs=NB * DC))
            kTp = esBC.enter_context(tc.tile_pool(name="kT", bufs=NB * DC))
            vp = esBC.enter_context(tc.tile_pool(name="vna", bufs=NB * 3))
            esB = ExitStack()
            pwB = esB.enter_context(tc.tile_pool(name="pwB", bufs=1))

            qT = project_T(qwT_d, HT, qb_sb, qTp, "qT", pwB)
            kT = project_T(kwT_d, HT, kb_sb, kTp, "kT", pwB)

            # v in natural layout [s, D] (no bias: folded into attn eviction)
            vna = []
            v_src = vwT_d.rearrange("(kc p) m -> p kc m", p=128)
            for half in range(2):
                wt = pwB.tile([128, DC, 512], F32R, name="pw", tag="pw")
                for kc in range(DC):
                    nc.sync.dma_start(
                        wt[:, kc, :], v_src[:, kc, half * 512:(half + 1) * 512])
                for b in range(NB):
                    if half == 0:
                        vna.append([vp.tile([pz, D], F32R, name="vna",
                                            tag="vna") for (o, pz) in SQ])
                    for j, (o, pz) in enumerate(SQ):
                        ps = pp.tile([128, 512], F32, name="psV", tag="pp")
                        for kc in range(DC):
                            nc.tensor.matmul(
                                ps[:pz, :],
                                HT[b][kc][:, o:o + pz],
                                wt[:, kc, :],
                                start=(kc == 0), stop=(kc == DC - 1))
                        nc.scalar.copy(
                            vna[b][j][:, half * 512:(half + 1) * 512],
                            ps[:pz, :])
            esB.close()    # qkv weights dead
            esA_HT.close()  # HT dead

            # ---------- stage C: attention ----------
            esC = ExitStack()
            probsp = esC.enter_context(tc.tile_pool(name="probs", bufs=4))
            probsTp = esC.enter_context(tc.tile_pool(name="probsT", bufs=4))
            esCD = ExitStack()
            attnTp = esCD.enter_context(
                tc.tile_pool(name="attnT", bufs=NB * DC, side="right"))
            attnT = []
            for b in range(NB):
                attnT.append([None] * DC)
                for h in range(H):
                    dc, po = h // 2, (h % 2) * 64
                    probs = []
                    for j, (o, pz) in enumerate(SQ):
                        sc = pp.tile([128, SE], F32, name="psS", tag="pp")
                        nc.tensor.matmul(
                            sc[:pz, :],
                            qT[b][dc][po:po + 64, o:o + pz],
                            kT[b][dc][po:po + 64, :],
                            start=True, stop=True)
                        pr = probsp.tile([pz, S], F32, name="pr", tag="pr")
                        sm = stat.tile([pz, 1], F32, name="sm", tag="sm")
                        nc.scalar.activation(pr[:], sc[:pz, 0:S], AF.Exp,
                                             accum_out=sm[:])
                        rc = stat.tile([pz, 1], F32, name="rc", tag="rc")
                        nc.vector.reciprocal(rc[:], sm[:])
                        nc.vector.tensor_scalar_mul(pr[:], pr[:], rc[:])
                        probs.append(pr)
                    pTs = []
                    for sj, (so, spz) in enumerate(SQ):
                        ps = pt.tile([spz, SE], F32, name="psPT", tag="pt")
                        for qj, (qo, qpz) in enumerate(SQ):
                            nc.tensor.transpose(
                                ps[:, qo:qo + qpz],
                                probs[qj][:, so:so + spz],
                                ident[:qpz, :qpz])
                        pT = probsTp.tile([spz, SE], F32R, name="pT",
                                          tag="pT")
                        nc.scalar.copy(pT[:], ps[:])
                        pTs.append(pT)
                    at = pp.tile([64, SE], F32, name="psA", tag="pp")
                    for sj, (so, spz) in enumerate(SQ):
                        nc.tensor.matmul(
                            at[:], vna[b][sj][:, h * 64:(h + 1) * 64],
                            pTs[sj][:],
                            start=(sj == 0), stop=(sj == 2))
                    if po == 0:
                        attnT[b][dc] = attnTp.tile([128, SE], F32R,
                                                   name="atT", tag="atT")
                    nc.scalar.activation(
                        attnT[b][dc][po:po + 64, :], at[:], AF.Identity,
                        bias=vb_sb[po:po + 64, dc:dc + 1])
            esC.close()   # probs, probsT dead
            esBC.close()  # qT, kT, vna dead

            # ---------- stage D: out_proj, residual, LN2, h2^T ----------
            esD2 = ExitStack()
            aoTp = esD2.enter_context(tc.tile_pool(name="aoT", bufs=NB * DC))
            esD = ExitStack()
            pwD = esD.enter_context(tc.tile_pool(name="pwD", bufs=1))
            aoT = project_T(owT_d, attnT, ob_sb, aoTp, "aoT", pwD, odt=F32)
            esD.close()   # ow weights dead
            esCD.close()  # attnT dead

            esDE = ExitStack()
            H2Tp = esDE.enter_context(
                tc.tile_pool(name="H2T", bufs=NB * DC, side="right"))
            esD3 = ExitStack()
            x1p = esD3.enter_context(
                tc.tile_pool(name="x1", bufs=3, side="right"))
            h2natp = esD3.enter_context(
                tc.tile_pool(name="h2nat", bufs=3, side="right"))
            H2T = []
            for b in range(NB):
                x1ts = []
                for j, (o, pz) in enumerate(SQ):
                    ps = pt.tile([pz, D], F32, name="psN", tag="pt")
                    for dc in range(DC):
                        nc.tensor.transpose(
                            ps[:, dc * 128:(dc + 1) * 128],
                            aoT[b][dc][:, o:o + pz], ident[:128, :128])
                    xres = xio.tile([pz, D], F32, name="xres", tag="xin")
                    nc.sync.dma_start(xres[:], x_d[b, o:o + pz, :])
                    x1t = x1p.tile([pz, D], F32, name="x1", tag="x1")
                    nc.vector.tensor_tensor(out=x1t[:], in0=ps[:], in1=xres[:],
                                            op=ALU.add)
                    nc.sync.dma_start(x1_scr[b, o:o + pz, :], x1t[:])
                    x1ts.append(x1t)
                h2ts = layer_norm(x1ts, h2natp)
                H2T.append(transpose_to_T(h2ts, H2Tp, g2_sb, b2_sb, "H2T"))
            esD2.close()  # aoT dead
            esD3.close()  # x1, h2nat dead

            # ---------- stage E: MLP in 2 batch groups ----------
            esE = ExitStack()
            w1p = esE.enter_context(tc.tile_pool(name="w1", bufs=2))
            w2p = esE.enter_context(tc.tile_pool(name="w2", bufs=2))
            h1Tp = esE.enter_context(tc.tile_pool(name="h1T", bufs=2 * FC))
            moTp = esE.enter_context(tc.tile_pool(name="moT", bufs=2 * DC))
            outnp = esE.enter_context(tc.tile_pool(name="outn", bufs=2))
            f1_src = f1wT_d.rearrange("(kc p) m -> p kc m", p=128)
            f2_src = f2wT_d.rearrange("(kc p) m -> p kc m", p=128)
            for grp in range(2):
                bs = [grp * 2, grp * 2 + 1]
                h1T = {b: [None] * FC for b in bs}
                for mc in range(FC):
                    w1t = w1p.tile([128, DC, 128], F32R, name="w1", tag="w1")
                    for kc in range(DC):
                        nc.sync.dma_start(
                            w1t[:, kc, :],
                            f1_src[:, kc, mc * 128:(mc + 1) * 128])
                    for b in bs:
                        ps = pp.tile([128, SE], F32, name="psF1", tag="pp")
                        for kc in range(DC):
                            nc.tensor.matmul(
                                ps[:], w1t[:, kc, :], H2T[b][kc][:],
                                start=(kc == 0), stop=(kc == DC - 1))
                        t = h1Tp.tile([128, SE], F32R, name="h1T",
                                      tag="h1T")
                        nc.scalar.activation(t[:], ps[:],
                                             AF.Gelu_apprx_sigmoid,
                                             bias=f1b_sb[:, mc:mc + 1])
                        h1T[b][mc] = t
                moT = {b: [None] * DC for b in bs}
                for mc in range(DC):
                    for kh in range(2):
                        w2t = w2p.tile([128, FC // 2, 128], F32R, name="w2",
                                       tag="w2")
                        for kc in range(FC // 2):
                            nc.sync.dma_start(
                                w2t[:, kc, :],
                                f2_src[:, kh * 16 + kc,
                                       mc * 128:(mc + 1) * 128])
                        if kh == 0:
                            ps2 = {b: pp.tile([128, SE], F32, name="psF2",
                                              tag="pp") for b in bs}
                        for b in bs:
                            for kc in range(FC // 2):
                                nc.tensor.matmul(
                                    ps2[b][:], w2t[:, kc, :],
                                    h1T[b][kh * 16 + kc][:],
                                    start=(kh == 0 and kc == 0),
                                    stop=(kh == 1 and kc == FC // 2 - 1))
                    for b in bs:
                        t = moTp.tile([128, SE], F32, name="moT", tag="moT")
                        nc.scalar.activation(t[:], ps2[b][:], AF.Identity,
                                             bias=f2b_sb[:, mc:mc + 1])
                        moT[b][mc] = t
                for b in bs:
                    for j, (o, pz) in enumerate(SQ):
                        ps = pt.tile([pz, D], F32, name="psO", tag="pt")
                        for dc in range(DC):
                            nc.tensor.transpose(
                                ps[:, dc * 128:(dc + 1) * 128],
                                moT[b][dc][:, o:o + pz], ident[:128, :128])
                        x1res = xio.tile([pz, D], F32, name="x1r", tag="xin")
                        nc.sync.dma_start(x1res[:], x1_scr[b, o:o + pz, :])
                        ot = outnp.tile([pz, D], F32, name="outn", tag="outn")
                        nc.vector.tensor_tensor(out=ot[:], in0=ps[:],
                                                in1=x1res[:], op=ALU.add)
                        nc.sync.dma_start(out_d[b, o:o + pz, :], ot[:])
            esE.close()
            esDE.close()

    nc.compile()
    return nc


_NC = None


def _get_nc():
    global _NC
    if _NC is None:
        _NC = build()
    return _NC


def _prep_inputs(inputs):
    f = lambda a: np.ascontiguousarray(np.asarray(a, dtype=np.float32))
    x = f(inputs["hidden_states"])
    shared = {
        "qwT": f(inputs["q_w"]).T * SCALE,
        "kwT": f(inputs["k_w"]).T,
        "vwT": f(inputs["v_w"]).T,
        "owT": f(inputs["o_w"]).T,
        "f1wT": f(inputs["fc1_w"]).T,
        "f2wT": f(inputs["fc2_w"]).T,
        "qb": f(inputs["q_b"]) * SCALE,
        "kb": f(inputs["k_b"]),
        "vb": f(inputs["v_b"]),
        "ob": f(inputs["o_b"]),
        "f1b": f(inputs["fc1_b"]),
        "f2b": f(inputs["fc2_b"]),
        "g1": f(inputs["ln1_g"]),
        "b1": f(inputs["ln1_b"]),
        "g2": f(inputs["ln2_g"]),
        "b2": f(inputs["ln2_b"]),
    }
    shared = {k: np.ascontiguousarray(v) for k, v in shared.items()}
    in_maps = []
    for c in range(N_CORES):
        m = dict(shared)
        m["x"] = np.ascontiguousarray(x[c * NB:(c + 1) * NB])
        in_maps.append(m)
    return in_maps


def run(inputs, trace=False):
    nc = _get_nc()
    in_maps = _prep_inputs(inputs)
    res = bass_utils.run_bass_kernel_spmd(
        nc, in_maps, core_ids=list(range(N_CORES)), trace=trace)
    out = np.concatenate([res.results[c]["out"] for c in range(N_CORES)],
                         axis=0)
    return out, res


def kernel(**inputs):
    out, _ = run(inputs, trace=False)
    return out


# revision 12
# speedup vs baseline: 1.5002x; 1.5002x over previous
"""ChineseCLIPVisionLayer on 8 trn2 NeuronCores.

Sharding: pure data-parallel over batch (B=32 -> 4 per core), zero
collectives. Weights are host-transposed and replicated to every core.

Per-core pipeline (all activations that feed matmuls live in transposed
layout [D, S] so the contraction dim sits on SBUF partitions):
  LN1 (natural) -> PE-transpose -> h^T
  q^T,k^T (transposed out), v (natural out)
  per head: scores = q_h^T.T @ k_h^T -> softmax (no max-sub; scores ~ +-3)
            probs -> PE-transpose -> probs^T
            attn^T = v_h.T @ probs^T   (+v_b via bias: softmax rows sum to 1)
  out_proj -> attn_out^T -> PE-transpose back + residual -> x1 (natural)
  LN2 -> h2^T ; MLP in 2 batch-groups (weights streamed once per group)
  quick-gelu == Gelu_apprx_sigmoid on ACT; fc2 out -> transpose + residual.
Matmuls run in float32r (full PE rate at N>=256; fp32 would be 4x slower).
Pools are stage-scoped (manually closed ExitStacks) to fit SBUF.
"""

from contextlib import ExitStack

import numpy as np

import concourse.bass as bass
import concourse.mybir as mybir
import concourse.tile as tile
from concourse import bacc, bass_utils
from concourse.masks import make_identity

N_CORES = 8
B, S, D = 32, 257, 1024
H, HD = 16, 64
FF = 4096
EPS = 1e-5
SCALE = HD ** -0.5
NB = B // N_CORES  # batch elems per core

F32 = mybir.dt.float32
F32R = mybir.dt.float32r
F16 = mybir.dt.float16
AF = mybir.ActivationFunctionType
ALU = mybir.AluOpType

# sequence chunks (partition-dim tiling of S=257)
SQ = [(0, 128), (128, 128), (256, 1)]
SE = 258  # free-dim padded length: fp32r matmul needs even moving/dst size
DC = D // 128   # 8 chunks of the model dim
FC = FF // 128  # 32 chunks of the ff dim


def r(ap):
    return ap.bitcast(F32R)


def build():
    nc = bacc.Bacc("TRN2", target_bir_lowering=False, debug=False,
                   num_devices=N_CORES)

    def din(name, shape, dt=F32):
        return nc.dram_tensor(name, shape, dt, kind="ExternalInput").ap()

    x_d = din("x", [NB, S, D])
    qwT_d = din("qwT", [D, D], F16)
    kwT_d = din("kwT", [D, D], F16)
    vwT_d = din("vwT", [D, D], F16)
    owT_d = din("owT", [D, D], F16)
    f1wT_d = din("f1wT", [D, FF], F16)
    f2wT_d = din("f2wT", [FF, D], F16)
    qb_d = din("qb", [D])
    kb_d = din("kb", [D])
    vb_d = din("vb", [D])
    ob_d = din("ob", [D])
    f1b_d = din("f1b", [FF])
    f2b_d = din("f2b", [D])
    g1_d = din("g1", [D])
    b1_d = din("b1", [D])
    g2_d = din("g2", [D])
    b2_d = din("b2", [D])
    out_d = nc.dram_tensor("out", [NB, S, D], F32, kind="ExternalOutput").ap()

    with tile.TileContext(nc) as tc:
        with ExitStack() as es:
            P = lambda name, bufs, **kw: es.enter_context(
                tc.tile_pool(name=name, bufs=bufs, **kw))
            const = P("const", 1)
            biasp = P("bias", 1)
            xio = P("xio", 2)
            stat = P("stat", 8)
            pp = P("pp", 8, space="PSUM")
            pt = pp
            dramp = P("dram", 1, space="DRAM")

            ident = const.tile([128, 128], F32)
            make_identity(nc, ident)
            ident16 = const.tile([128, 128], F16)
            make_identity(nc, ident16)
            epsc = const.tile([128, 1], F32)
            nc.vector.memset(epsc[:], EPS)

            def load_bias(dram, n):
                t = biasp.tile([128, n // 128], F32, name=f"bias_{dram.name}")
                nc.sync.dma_start(t[:], dram.rearrange("(c p) -> p c", p=128))
                return t

            qb_sb = load_bias(qb_d, D)
            kb_sb = load_bias(kb_d, D)
            vb_sb = load_bias(vb_d, D)
            ob_sb = load_bias(ob_d, D)
            f1b_sb = load_bias(f1b_d, FF)
            f2b_sb = load_bias(f2b_d, D)
            g1_sb = load_bias(g1_d, D)
            b1_sb = load_bias(b1_d, D)
            g2_sb = load_bias(g2_d, D)
            b2_sb = load_bias(b2_d, D)

            x1_scr = dramp.tile([NB, S, D], F32)

            def layer_norm(src_tiles, hpool):
                """src_tiles: 3 natural tiles [(pz, D)]; returns normalized
                (x-mu)*rstd tiles (gamma/beta applied at transpose evict)."""
                out_tiles = []
                for j, (o, pz) in enumerate(SQ):
                    xt = src_tiles[j]
                    st = stat.tile([pz, 2, 6], F32, name="st", tag="st")
                    nc.vector.bn_stats(st[:, 0, :], xt[:, 0:512])
                    nc.vector.bn_stats(st[:, 1, :], xt[:, 512:1024])
                    mv = stat.tile([pz, 2], F32, name="mv", tag="mv")
                    nc.vector.bn_aggr(mv[:], st[:])
                    rstd = stat.tile([pz, 1], F32, name="rstd", tag="rstd")
                    nc.scalar.activation(rstd[:], mv[:, 1:2], AF.Sqrt,
                                         bias=epsc[:pz, :])
                    nc.vector.reciprocal(rstd[:], rstd[:])
                    ht = hpool.tile([pz, D], F32, name="hn", tag="hn")
                    nc.vector.tensor_scalar(
                        out=ht[:], in0=xt[:], scalar1=mv[:, 0:1],
                        scalar2=rstd[:], op0=ALU.subtract, op1=ALU.mult)
                    out_tiles.append(ht)
                return out_tiles

            def transpose_to_T(nat_tiles, dst_pool, g_sb, bt_sb, tag):
                """nat tiles [(pz, D)] -> 8 tiles [128, S] of the transpose,
                evicted with per-partition scale g and bias bt."""
                outs = []
                for dc in range(DC):
                    ps = pt.tile([128, SE], F32, name="psT", tag="pp")
                    for j, (o, pz) in enumerate(SQ):
                        nc.tensor.transpose(
                            ps[:, o:o + pz],
                            nat_tiles[j][:, dc * 128:(dc + 1) * 128],
                            ident[:pz, :pz])
                    t = dst_pool.tile([128, SE], F16, name=f"{tag}", tag=tag)
                    if g_sb is None:
                        nc.scalar.copy(t[:], ps[:])
                    else:
                        nc.scalar.activation(
                            t[:], ps[:], AF.Identity,
                            bias=bt_sb[:, dc:dc + 1], scale=g_sb[:, dc:dc + 1])
                    outs.append(t)
                return outs

            def project_T(wT_dram, rhs_per_b, bias_sb, dst_pool, tag, wpool,
                          func=AF.Identity, odt=F16):
                """y^T = wT.T @ rhs (+bias) for every batch elem.
                Streams wT in two m-halves of [128, DC, 512] to bound SBUF.
                Returns outs[b][mc] tiles [128, S]."""
                src = wT_dram.rearrange("(kc p) m -> p kc m", p=128)
                outs = [[None] * DC for _ in range(NB)]
                for half in range(2):
                    wt = wpool.tile([128, DC, 512], F16, name="pw", tag="pw")
                    nc.sync.dma_start(
                        wt[:], src[:, :, half * 512:(half + 1) * 512])
                    for b in range(NB):
                        for ml in range(4):
                            mc = half * 4 + ml
                            ps = pp.tile([128, SE], F32, name="psP", tag="pp")
                            for kc in range(DC):
                                nc.tensor.matmul(
                                    ps[:],
                                    wt[:, kc, ml * 128:(ml + 1) * 128],
                                    rhs_per_b[b][kc][:],
                                    start=(kc == 0), stop=(kc == DC - 1))
                            t = dst_pool.tile([128, SE], odt, name=tag,
                                              tag=tag)
                            nc.scalar.activation(t[:], ps[:], func,
                                                 bias=bias_sb[:, mc:mc + 1])
                            outs[b][mc] = t
                return outs

            # ---------- stage A: load x, LN1, h^T ----------
            esA_HT = ExitStack()
            HTp = esA_HT.enter_context(
                tc.tile_pool(name="HT", bufs=NB * DC, side="right"))
            esA = ExitStack()
            hnat = esA.enter_context(
                tc.tile_pool(name="hnat", bufs=3, side="right"))
            HT = []
            for b in range(NB):
                xts = []
                for j, (o, pz) in enumerate(SQ):
                    xt = xio.tile([pz, D], F32, name="xin", tag="xin")
                    nc.sync.dma_start(xt[:], x_d[b, o:o + pz, :])
                    xts.append(xt)
                hts = layer_norm(xts, hnat)
                HT.append(transpose_to_T(hts, HTp, g1_sb, b1_sb, "HT"))

            # ---------- stage B: QKV ----------
            esA.close()  # hnat dead
            esBC = ExitStack()
            qTp = esBC.enter_context(tc.tile_pool(name="qT", bufs=NB * DC))
            kTp = esBC.enter_context(tc.tile_pool(name="kT", bufs=NB * DC))
            vp = esBC.enter_context(tc.tile_pool(name="vna", bufs=NB * 3))
            esB = ExitStack()
            pwB = esB.enter_context(tc.tile_pool(name="pwB", bufs=2))

            qT = project_T(qwT_d, HT, qb_sb, qTp, "qT", pwB)
            kT = project_T(kwT_d, HT, kb_sb, kTp, "kT", pwB)

            # v in natural layout [s, D] (no bias: folded into attn eviction)
            vna = []
            v_src = vwT_d.rearrange("(kc p) m -> p kc m", p=128)
            for half in range(2):
                wt = pwB.tile([128, DC, 512], F16, name="pw", tag="pw")
                nc.sync.dma_start(
                    wt[:], v_src[:, :, half * 512:(half + 1) * 512])
                for b in range(NB):
                    if half == 0:
                        vna.append([vp.tile([pz, D], F16, name="vna",
                                            tag="vna") for (o, pz) in SQ])
                    for j, (o, pz) in enumerate(SQ):
                        ps = pp.tile([128, 512], F32, name="psV", tag="pp")
                        for kc in range(DC):
                            nc.tensor.matmul(
                                ps[:pz, :],
                                HT[b][kc][:, o:o + pz],
                                wt[:, kc, :],
                                start=(kc == 0), stop=(kc == DC - 1))
                        nc.scalar.copy(
                            vna[b][j][:, half * 512:(half + 1) * 512],
                            ps[:pz, :])
            esB.close()    # qkv weights dead
            esA_HT.close()  # HT dead

            # ---------- stage C: attention ----------
            esC = ExitStack()
            probsp = esC.enter_context(tc.tile_pool(name="probs", bufs=4))
            probsTp = esC.enter_context(tc.tile_pool(name="probsT", bufs=4))
            esCD = ExitStack()
            attnTp = esCD.enter_context(
                tc.tile_pool(name="attnT", bufs=NB * DC, side="right"))
            attnT = []
            for b in range(NB):
                attnT.append([None] * DC)
                for h in range(H):
                    dc, po = h // 2, (h % 2) * 64
                    probs = []
                    for j, (o, pz) in enumerate(SQ):
                        sc = pp.tile([128, SE], F32, name="psS", tag="pp")
                        nc.tensor.matmul(
                            sc[:pz, :],
                            qT[b][dc][po:po + 64, o:o + pz],
                            kT[b][dc][po:po + 64, :],
                            start=True, stop=True)
                        pr = probsp.tile([pz, S], F16, name="pr", tag="pr")
                        sm = stat.tile([pz, 1], F32, name="sm", tag="sm")
                        nc.scalar.activation(pr[:], sc[:pz, 0:S], AF.Exp,
                                             accum_out=sm[:])
                        rc = stat.tile([pz, 1], F32, name="rc", tag="rc")
                        nc.vector.reciprocal(rc[:], sm[:])
                        nc.vector.tensor_scalar_mul(pr[:], pr[:], rc[:])
                        probs.append(pr)
                    pTs = []
                    for sj, (so, spz) in enumerate(SQ):
                        ps = pt.tile([spz, SE], F16, name="psPT", tag="pp")
                        for qj, (qo, qpz) in enumerate(SQ):
                            nc.tensor.transpose(
                                ps[:, qo:qo + qpz],
                                probs[qj][:, so:so + spz],
                                ident16[:qpz, :qpz])
                        pT = probsTp.tile([spz, SE], F16, name="pT",
                                          tag="pT")
                        nc.scalar.copy(pT[:], ps[:])
                        pTs.append(pT)
                    at = pp.tile([64, SE], F32, name="psA", tag="pp")
                    for sj, (so, spz) in enumerate(SQ):
                        nc.tensor.matmul(
                            at[:], vna[b][sj][:, h * 64:(h + 1) * 64],
                            pTs[sj][:],
                            start=(sj == 0), stop=(sj == 2))
                    if po == 0:
                        attnT[b][dc] = attnTp.tile([128, SE], F16,
                                                   name="atT", tag="atT")
                    nc.scalar.activation(
                        attnT[b][dc][po:po + 64, :], at[:], AF.Identity,
                        bias=vb_sb[po:po + 64, dc:dc + 1])
            esC.close()   # probs, probsT dead
            esBC.close()  # qT, kT, vna dead

            # ---------- stage D: out_proj, residual, LN2, h2^T ----------
            esD2 = ExitStack()
            aoTp = esD2.enter_context(tc.tile_pool(name="aoT", bufs=NB * DC))
            esD = ExitStack()
            pwD = esD.enter_context(tc.tile_pool(name="pwD", bufs=2))
            aoT = project_T(owT_d, attnT, ob_sb, aoTp, "aoT", pwD, odt=F32)
            esD.close()   # ow weights dead
            esCD.close()  # attnT dead

            esDE = ExitStack()
            H2Tp = esDE.enter_context(
                tc.tile_pool(name="H2T", bufs=NB * DC, side="right"))
            esD3 = ExitStack()
            x1p = esD3.enter_context(
                tc.tile_pool(name="x1", bufs=3, side="right"))
            h2natp = esD3.enter_context(
                tc.tile_pool(name="h2nat", bufs=3, side="right"))
            H2T = []
            for b in range(NB):
                x1ts = []
                for j, (o, pz) in enumerate(SQ):
                    xres = xio.tile([pz, D], F32, name="xres", tag="xin")
                    nc.sync.dma_start(xres[:], x_d[b, o:o + pz, :])
                    x1t = x1p.tile([pz, D], F32, name="x1", tag="x1")
                    for hf in range(2):
                        ps = pt.tile([pz, 512], F32, name="psN", tag="pp")
                        for dl in range(4):
                            dc = hf * 4 + dl
                            nc.tensor.transpose(
                                ps[:, dl * 128:(dl + 1) * 128],
                                aoT[b][dc][:, o:o + pz], ident[:128, :128])
                        nc.vector.tensor_tensor(
                            out=x1t[:, hf * 512:(hf + 1) * 512], in0=ps[:],
                            in1=xres[:, hf * 512:(hf + 1) * 512], op=ALU.add)
                    nc.sync.dma_start(x1_scr[b, o:o + pz, :], x1t[:])
                    x1ts.append(x1t)
                h2ts = layer_norm(x1ts, h2natp)
                H2T.append(transpose_to_T(h2ts, H2Tp, g2_sb, b2_sb, "H2T"))
            esD2.close()  # aoT dead
            esD3.close()  # x1, h2nat dead

            # ---------- stage E: MLP in 2 batch groups ----------
            esE = ExitStack()
            w1p = esE.enter_context(tc.tile_pool(name="w1", bufs=2))
            w2p = esE.enter_context(tc.tile_pool(name="w2", bufs=2))
            h1Tp = esE.enter_context(tc.tile_pool(name="h1T", bufs=2 * FC))
            moTp = esE.enter_context(tc.tile_pool(name="moT", bufs=2 * DC))
            outnp = esE.enter_context(tc.tile_pool(name="outn", bufs=2))
            f1_src = f1wT_d.rearrange("(kc p) m -> p kc m", p=128)
            f2_src = f2wT_d.rearrange("(kc p) m -> p kc m", p=128)
            for grp in range(2):
                bs = [grp * 2, grp * 2 + 1]
                h1T = {b: [None] * FC for b in bs}
                for mc in range(FC):
                    w1t = w1p.tile([128, DC, 128], F16, name="w1", tag="w1")
                    nc.sync.dma_start(
                        w1t[:], f1_src[:, :, mc * 128:(mc + 1) * 128])
                    for b in bs:
                        ps = pp.tile([128, SE], F32, name="psF1", tag="pp")
                        for kc in range(DC):
                            nc.tensor.matmul(
                                ps[:], w1t[:, kc, :], H2T[b][kc][:],
                                start=(kc == 0), stop=(kc == DC - 1))
                        t = h1Tp.tile([128, SE], F16, name="h1T",
                                      tag="h1T")
                        nc.scalar.activation(t[:], ps[:],
                                             AF.Gelu_apprx_sigmoid,
                                             bias=f1b_sb[:, mc:mc + 1])
                        h1T[b][mc] = t
                moT = {b: [None] * DC for b in bs}
                for mc in range(DC):
                    for kh in range(2):
                        w2t = w2p.tile([128, FC // 2, 128], F16, name="w2",
                                       tag="w2")
                        nc.sync.dma_start(
                            w2t[:], f2_src[:, kh * 16:(kh + 1) * 16,
                                           mc * 128:(mc + 1) * 128])
                        if kh == 0:
                            ps2 = {b: pp.tile([128, SE], F32, name="psF2",
                                              tag="pp") for b in bs}
                        for b in bs:
                            for kc in range(FC // 2):
                                nc.tensor.matmul(
                                    ps2[b][:], w2t[:, kc, :],
                                    h1T[b][kh * 16 + kc][:],
                                    start=(kh == 0 and kc == 0),
                                    stop=(kh == 1 and kc == FC // 2 - 1))
                    for b in bs:
                        t = moTp.tile([128, SE], F32, name="moT", tag="moT")
                        nc.scalar.activation(t[:], ps2[b][:], AF.Identity,
                                             bias=f2b_sb[:, mc:mc + 1])
                        moT[b][mc] = t
                for b in bs:
                    for j, (o, pz) in enumerate(SQ):
                        x1res = xio.tile([pz, D], F32, name="x1r", tag="xin")
                        nc.sync.dma_start(x1res[:], x1_scr[b, o:o + pz, :])
                        ot = outnp.tile([pz, D], F32, name="outn", tag="outn")
                        for hf in range(2):
                            ps = pt.tile([pz, 512], F32, name="psO", tag="pp")
                            for dl in range(4):
                                dc = hf * 4 + dl
                                nc.tensor.transpose(
                                    ps[:, dl * 128:(dl + 1) * 128],
                                    moT[b][dc][:, o:o + pz], ident[:128, :128])
                            nc.vector.tensor_tensor(
                                out=ot[:, hf * 512:(hf + 1) * 512], in0=ps[:],
                                in1=x1res[:, hf * 512:(hf + 1) * 512],
                                op=ALU.add)
                        nc.sync.dma_start(out_d[b, o:o + pz, :], ot[:])
            esE.close()
            esDE.close()

    nc.compile()
    return nc


_NC = None


def _get_nc():
    global _NC
    if _NC is None:
        _NC = build()
    return _NC


def _prep_inputs(inputs):
    f = lambda a: np.ascontiguousarray(np.asarray(a, dtype=np.float32))
    x = f(inputs["hidden_states"])
    h = lambda a: np.ascontiguousarray(a.astype(np.float16))
    shared = {
        "qwT": h(f(inputs["q_w"]).T * SCALE),
        "kwT": h(f(inputs["k_w"]).T),
        "vwT": h(f(inputs["v_w"]).T),
        "owT": h(f(inputs["o_w"]).T),
        "f1wT": h(f(inputs["fc1_w"]).T),
        "f2wT": h(f(inputs["fc2_w"]).T),
        "qb": f(inputs["q_b"]) * SCALE,
        "kb": f(inputs["k_b"]),
        "vb": f(inputs["v_b"]),
        "ob": f(inputs["o_b"]),
        "f1b": f(inputs["fc1_b"]),
        "f2b": f(inputs["fc2_b"]),
        "g1": f(inputs["ln1_g"]),
        "b1": f(inputs["ln1_b"]),
        "g2": f(inputs["ln2_g"]),
        "b2": f(inputs["ln2_b"]),
    }
    shared = {k: np.ascontiguousarray(v) for k, v in shared.items()}
    in_maps = []
    for c in range(N_CORES):
        m = dict(shared)
        m["x"] = np.ascontiguousarray(x[c * NB:(c + 1) * NB])
        in_maps.append(m)
    return in_maps


def run(inputs, trace=False):
    nc = _get_nc()
    in_maps = _prep_inputs(inputs)
    res = bass_utils.run_bass_kernel_spmd(
        nc, in_maps, core_ids=list(range(N_CORES)), trace=trace)
    out = np.concatenate([res.results[c]["out"] for c in range(N_CORES)],
                         axis=0)
    return out, res


def kernel(**inputs):
    out, _ = run(inputs, trace=False)
    return out


# revision 13
# speedup vs baseline: 1.7822x; 1.1880x over previous
"""ChineseCLIPVisionLayer on 8 trn2 NeuronCores.

Sharding: pure data-parallel over batch (B=32 -> 4 per core), zero
collectives. Weights are host-transposed and replicated to every core.

Per-core pipeline (all activations that feed matmuls live in transposed
layout [D, S] so the contraction dim sits on SBUF partitions):
  LN1 (natural) -> PE-transpose -> h^T
  q^T,k^T (transposed out), v (natural out)
  per head: scores = q_h^T.T @ k_h^T -> softmax (no max-sub; scores ~ +-3)
            probs -> PE-transpose -> probs^T
            attn^T = v_h.T @ probs^T   (+v_b via bias: softmax rows sum to 1)
  out_proj -> attn_out^T -> PE-transpose back + residual -> x1 (natural)
  LN2 -> h2^T ; MLP in 2 batch-groups (weights streamed once per group)
  quick-gelu == Gelu_apprx_sigmoid on ACT; fc2 out -> transpose + residual.
Matmuls run in float32r (full PE rate at N>=256; fp32 would be 4x slower).
Pools are stage-scoped (manually closed ExitStacks) to fit SBUF.
"""

from contextlib import ExitStack

import numpy as np

import concourse.bass as bass
import concourse.mybir as mybir
import concourse.tile as tile
from concourse import bacc, bass_utils
from concourse.masks import make_identity

N_CORES = 8
B, S, D = 32, 257, 1024
H, HD = 16, 64
FF = 4096
EPS = 1e-5
SCALE = HD ** -0.5
NB = B // N_CORES  # batch elems per core

F32 = mybir.dt.float32
F32R = mybir.dt.float32r
F16 = mybir.dt.float16
AF = mybir.ActivationFunctionType
ALU = mybir.AluOpType

# sequence chunks (partition-dim tiling of S=257)
SQ = [(0, 128), (128, 128), (256, 1)]
SE = 258  # free-dim padded length: fp32r matmul needs even moving/dst size
DC = D // 128   # 8 chunks of the model dim
FC = FF // 128  # 32 chunks of the ff dim


def r(ap):
    return ap.bitcast(F32R)


def build():
    nc = bacc.Bacc("TRN2", target_bir_lowering=False, debug=False,
                   num_devices=N_CORES)

    def din(name, shape, dt=F32):
        return nc.dram_tensor(name, shape, dt, kind="ExternalInput").ap()

    x_d = din("x", [NB, S, D])
    qwT_d = din("qwT", [D, D], F16)
    kwT_d = din("kwT", [D, D], F16)
    vwT_d = din("vwT", [D, D], F16)
    owT_d = din("owT", [D, D], F16)
    f1wT_d = din("f1wT", [D, FF], F16)
    f2wT_d = din("f2wT", [FF, D], F16)
    qb_d = din("qb", [D])
    kb_d = din("kb", [D])
    ob_d = din("ob", [D])
    f1b_d = din("f1b", [FF])
    f2b_d = din("f2b", [D])
    g1_d = din("g1", [D])
    b1_d = din("b1", [D])
    g2_d = din("g2", [D])
    b2_d = din("b2", [D])
    out_d = nc.dram_tensor("out", [NB, S, D], F32, kind="ExternalOutput").ap()

    with tile.TileContext(nc) as tc:
        with ExitStack() as es:
            P = lambda name, bufs, **kw: es.enter_context(
                tc.tile_pool(name=name, bufs=bufs, **kw))
            const = P("const", 1)
            biasp = P("bias", 1)
            xio = P("xio", 2)
            stat = P("stat", 8)
            pp = P("pp", 8, space="PSUM")
            pt = pp
            dramp = P("dram", 1, space="DRAM")

            ident = const.tile([128, 128], F32)
            make_identity(nc, ident)
            ident16 = const.tile([128, 128], F16)
            make_identity(nc, ident16)
            ones16 = const.tile([128, 1], F16)
            nc.vector.memset(ones16[:], 1.0)
            epsc = const.tile([128, 1], F32)
            nc.vector.memset(epsc[:], EPS)

            def load_bias(dram, n):
                t = biasp.tile([128, n // 128], F32, name=f"bias_{dram.name}")
                nc.sync.dma_start(t[:], dram.rearrange("(c p) -> p c", p=128))
                return t

            qb_sb = load_bias(qb_d, D)
            kb_sb = load_bias(kb_d, D)
            ob_sb = load_bias(ob_d, D)
            f1b_sb = load_bias(f1b_d, FF)
            f2b_sb = load_bias(f2b_d, D)
            g1_sb = load_bias(g1_d, D)
            b1_sb = load_bias(b1_d, D)
            g2_sb = load_bias(g2_d, D)
            b2_sb = load_bias(b2_d, D)

            x1_scr = dramp.tile([NB, S, D], F32)

            def layer_norm(src_tiles, hpool):
                """src_tiles: 3 natural tiles [(pz, D)]; returns normalized
                (x-mu)*rstd tiles (gamma/beta applied at transpose evict)."""
                out_tiles = []
                for j, (o, pz) in enumerate(SQ):
                    xt = src_tiles[j]
                    st = stat.tile([pz, 2, 6], F32, name="st", tag="st")
                    nc.vector.bn_stats(st[:, 0, :], xt[:, 0:512])
                    nc.vector.bn_stats(st[:, 1, :], xt[:, 512:1024])
                    mv = stat.tile([pz, 2], F32, name="mv", tag="mv")
                    nc.vector.bn_aggr(mv[:], st[:])
                    rstd = stat.tile([pz, 1], F32, name="rstd", tag="rstd")
                    nc.scalar.activation(rstd[:], mv[:, 1:2], AF.Sqrt,
                                         bias=epsc[:pz, :])
                    nc.vector.reciprocal(rstd[:], rstd[:])
                    ht = hpool.tile([pz, D], F32, name="hn", tag="hn")
                    nc.vector.tensor_scalar(
                        out=ht[:], in0=xt[:], scalar1=mv[:, 0:1],
                        scalar2=rstd[:], op0=ALU.subtract, op1=ALU.mult)
                    out_tiles.append(ht)
                return out_tiles

            def transpose_to_T(nat_tiles, dst_pool, g_sb, bt_sb, tag):
                """nat tiles [(pz, D)] -> 8 tiles [128, S] of the transpose,
                evicted with per-partition scale g and bias bt."""
                outs = []
                for dc in range(DC):
                    ps = pt.tile([128, SE], F32, name="psT", tag="pp")
                    for j, (o, pz) in enumerate(SQ):
                        nc.tensor.transpose(
                            ps[:, o:o + pz],
                            nat_tiles[j][:, dc * 128:(dc + 1) * 128],
                            ident[:pz, :pz])
                    t = dst_pool.tile([128, SE], F16, name=f"{tag}", tag=tag)
                    nc.vector.tensor_scalar(
                        out=t[:], in0=ps[:], scalar1=g_sb[:, dc:dc + 1],
                        scalar2=bt_sb[:, dc:dc + 1], op0=ALU.mult, op1=ALU.add)
                    outs.append(t)
                return outs

            def project_T(wT_dram, rhs_per_b, bias_sb, dst_pool, tag, wpool,
                          func=AF.Identity, odt=F16):
                """y^T = wT.T @ rhs (+bias) for every batch elem.
                Streams wT in two m-halves of [128, DC, 512] to bound SBUF.
                Returns outs[b][mc] tiles [128, S]."""
                src = wT_dram.rearrange("(kc p) m -> p kc m", p=128)
                outs = [[None] * DC for _ in range(NB)]
                for half in range(2):
                    wt = wpool.tile([128, DC, 512], F16, name="pw", tag="pw")
                    nc.sync.dma_start(
                        wt[:], src[:, :, half * 512:(half + 1) * 512])
                    for b in range(NB):
                        for ml in range(4):
                            mc = half * 4 + ml
                            ps = pp.tile([128, SE], F32, name="psP", tag="pp")
                            for kc in range(DC):
                                nc.tensor.matmul(
                                    ps[:],
                                    wt[:, kc, ml * 128:(ml + 1) * 128],
                                    rhs_per_b[b][kc][:],
                                    start=(kc == 0), stop=(kc == DC - 1))
                            t = dst_pool.tile([128, SE], odt, name=tag,
                                              tag=tag)
                            if func is AF.Identity:
                                nc.vector.tensor_scalar_add(
                                    t[:], ps[:], bias_sb[:, mc:mc + 1])
                            else:
                                nc.scalar.activation(t[:], ps[:], func,
                                                     bias=bias_sb[:, mc:mc + 1])
                            outs[b][mc] = t
                return outs

            # ---------- stage A: load x, LN1, h^T ----------
            esA_HT = ExitStack()
            HTp = esA_HT.enter_context(
                tc.tile_pool(name="HT", bufs=NB * DC, side="right"))
            esA = ExitStack()
            hnat = esA.enter_context(
                tc.tile_pool(name="hnat", bufs=3, side="right"))
            HT = []
            for b in range(NB):
                xts = []
                for j, (o, pz) in enumerate(SQ):
                    xt = xio.tile([pz, D], F32, name="xin", tag="xin")
                    nc.sync.dma_start(xt[:], x_d[b, o:o + pz, :])
                    xts.append(xt)
                hts = layer_norm(xts, hnat)
                HT.append(transpose_to_T(hts, HTp, g1_sb, b1_sb, "HT"))

            # ---------- stage B: QKV ----------
            esA.close()  # hnat dead
            esBC = ExitStack()
            qTp = esBC.enter_context(tc.tile_pool(name="qT", bufs=NB * DC))
            kTp = esBC.enter_context(tc.tile_pool(name="kT", bufs=NB * DC))
            vp = esBC.enter_context(tc.tile_pool(name="vna", bufs=NB * 3))
            esB = ExitStack()
            pwB = esB.enter_context(tc.tile_pool(name="pwB", bufs=2))

            qT = project_T(qwT_d, HT, qb_sb, qTp, "qT", pwB)
            kT = project_T(kwT_d, HT, kb_sb, kTp, "kT", pwB)

            # v in natural layout [s, D] (no bias: folded into attn eviction)
            vna = []
            v_src = vwT_d.rearrange("(kc p) m -> p kc m", p=128)
            for half in range(2):
                wt = pwB.tile([128, DC, 512], F16, name="pw", tag="pw")
                nc.sync.dma_start(
                    wt[:], v_src[:, :, half * 512:(half + 1) * 512])
                for b in range(NB):
                    if half == 0:
                        vna.append([vp.tile([pz, D], F16, name="vna",
                                            tag="vna") for (o, pz) in SQ])
                    for j, (o, pz) in enumerate(SQ):
                        ps = pp.tile([128, 512], F32, name="psV", tag="pp")
                        for kc in range(DC):
                            nc.tensor.matmul(
                                ps[:pz, :],
                                HT[b][kc][:, o:o + pz],
                                wt[:, kc, :],
                                start=(kc == 0), stop=(kc == DC - 1))
                        nc.vector.tensor_copy(
                            vna[b][j][:, half * 512:(half + 1) * 512],
                            ps[:pz, :])
            esB.close()    # qkv weights dead
            esA_HT.close()  # HT dead

            # ---------- stage C: attention ----------
            # scoresT = k_h @ q_h^T directly (no probs transpose); softmax
            # denominator via ones-matmul column sums; normalization fused
            # into the DVE eviction of attn^T; v_b folded into o_b on host.
            esC = ExitStack()
            probsTp = esC.enter_context(tc.tile_pool(name="probsT", bufs=8))
            rcp = esC.enter_context(tc.tile_pool(name="rcp", bufs=4))
            esCD = ExitStack()
            attnTp = esCD.enter_context(
                tc.tile_pool(name="attnT", bufs=NB * DC, side="right"))
            attnT = []
            for b in range(NB):
                attnT.append([None] * DC)
                for h in range(H):
                    dc, po = h // 2, (h % 2) * 64
                    pTs = []
                    csum = pp.tile([1, SE], F32, name="psCS", tag="pp")
                    for sj, (so, spz) in enumerate(SQ):
                        scT = pp.tile([128, SE], F32, name="psS", tag="pp")
                        nc.tensor.matmul(
                            scT[:spz, :],
                            kT[b][dc][po:po + 64, so:so + spz],
                            qT[b][dc][po:po + 64, :],
                            start=True, stop=True)
                        pT = probsTp.tile([spz, SE], F16, name="pT", tag="pT")
                        nc.scalar.activation(pT[:], scT[:spz, :], AF.Exp)
                        pTs.append(pT)
                        nc.tensor.matmul(
                            csum[:], ones16[:spz, :], pTs[sj][:],
                            start=(sj == 0), stop=(sj == 2))
                    rc = rcp.tile([1, SE], F32, name="rc", tag="rc")
                    nc.vector.reciprocal(rc[:], csum[:])
                    rcb = rcp.tile([64, SE], F32, name="rcb", tag="rcb")
                    nc.gpsimd.partition_broadcast(rcb[:], rc[:])
                    at = pp.tile([64, SE], F32, name="psA", tag="pp")
                    for sj, (so, spz) in enumerate(SQ):
                        nc.tensor.matmul(
                            at[:], vna[b][sj][:, h * 64:(h + 1) * 64],
                            pTs[sj][:],
                            start=(sj == 0), stop=(sj == 2))
                    if po == 0:
                        attnT[b][dc] = attnTp.tile([128, SE], F16,
                                                   name="atT", tag="atT")
                    nc.vector.tensor_tensor(
                        out=attnT[b][dc][po:po + 64, :], in0=at[:],
                        in1=rcb[:], op=ALU.mult)
            esC.close()   # probsT, rcp dead
            esBC.close()  # qT, kT, vna dead

            # ---------- stage D: out_proj, residual, LN2, h2^T ----------
            esD2 = ExitStack()
            aoTp = esD2.enter_context(tc.tile_pool(name="aoT", bufs=NB * DC))
            esD = ExitStack()
            pwD = esD.enter_context(tc.tile_pool(name="pwD", bufs=2))
            aoT = project_T(owT_d, attnT, ob_sb, aoTp, "aoT", pwD, odt=F32)
            esD.close()   # ow weights dead
            esCD.close()  # attnT dead

            esDE = ExitStack()
            H2Tp = esDE.enter_context(
                tc.tile_pool(name="H2T", bufs=NB * DC, side="right"))
            esD3 = ExitStack()
            x1p = esD3.enter_context(
                tc.tile_pool(name="x1", bufs=3, side="right"))
            h2natp = esD3.enter_context(
                tc.tile_pool(name="h2nat", bufs=3, side="right"))
            H2T = []
            for b in range(NB):
                x1ts = []
                for j, (o, pz) in enumerate(SQ):
                    xres = xio.tile([pz, D], F32, name="xres", tag="xin")
                    nc.sync.dma_start(xres[:], x_d[b, o:o + pz, :])
                    x1t = x1p.tile([pz, D], F32, name="x1", tag="x1")
                    for hf in range(2):
                        ps = pt.tile([pz, 512], F32, name="psN", tag="pp")
                        for dl in range(4):
                            dc = hf * 4 + dl
                            nc.tensor.transpose(
                                ps[:, dl * 128:(dl + 1) * 128],
                                aoT[b][dc][:, o:o + pz], ident[:128, :128])
                        nc.vector.tensor_tensor(
                            out=x1t[:, hf * 512:(hf + 1) * 512], in0=ps[:],
                            in1=xres[:, hf * 512:(hf + 1) * 512], op=ALU.add)
                    nc.sync.dma_start(x1_scr[b, o:o + pz, :], x1t[:])
                    x1ts.append(x1t)
                h2ts = layer_norm(x1ts, h2natp)
                H2T.append(transpose_to_T(h2ts, H2Tp, g2_sb, b2_sb, "H2T"))
            esD2.close()  # aoT dead
            esD3.close()  # x1, h2nat dead

            # ---------- stage E: MLP in 2 batch groups ----------
            esE = ExitStack()
            w1p = esE.enter_context(tc.tile_pool(name="w1", bufs=2))
            w2p = esE.enter_context(tc.tile_pool(name="w2", bufs=2))
            h1Tp = esE.enter_context(tc.tile_pool(name="h1T", bufs=2 * FC))
            moTp = esE.enter_context(tc.tile_pool(name="moT", bufs=2 * DC))
            outnp = esE.enter_context(tc.tile_pool(name="outn", bufs=2))
            f1_src = f1wT_d.rearrange("(kc p) m -> p kc m", p=128)
            f2_src = f2wT_d.rearrange("(kc p) m -> p kc m", p=128)
            for grp in range(2):
                bs = [grp * 2, grp * 2 + 1]
                h1T = {b: [None] * FC for b in bs}
                for mc in range(FC):
                    w1t = w1p.tile([128, DC, 128], F16, name="w1", tag="w1")
                    nc.sync.dma_start(
                        w1t[:], f1_src[:, :, mc * 128:(mc + 1) * 128])
                    for b in bs:
                        ps = pp.tile([128, SE], F32, name="psF1", tag="pp")
                        for kc in range(DC):
                            nc.tensor.matmul(
                                ps[:], w1t[:, kc, :], H2T[b][kc][:],
                                start=(kc == 0), stop=(kc == DC - 1))
                        t = h1Tp.tile([128, SE], F16, name="h1T",
                                      tag="h1T")
                        nc.scalar.activation(t[:], ps[:],
                                             AF.Gelu_apprx_sigmoid,
                                             bias=f1b_sb[:, mc:mc + 1])
                        h1T[b][mc] = t
                moT = {b: [None] * DC for b in bs}
                for mc in range(DC):
                    for kh in range(2):
                        w2t = w2p.tile([128, FC // 2, 128], F16, name="w2",
                                       tag="w2")
                        nc.sync.dma_start(
                            w2t[:], f2_src[:, kh * 16:(kh + 1) * 16,
                                           mc * 128:(mc + 1) * 128])
                        if kh == 0:
                            ps2 = {b: pp.tile([128, SE], F32, name="psF2",
                                              tag="pp") for b in bs}
                        for b in bs:
                            for kc in range(FC // 2):
                                nc.tensor.matmul(
                                    ps2[b][:], w2t[:, kc, :],
                                    h1T[b][kh * 16 + kc][:],
                                    start=(kh == 0 and kc == 0),
                                    stop=(kh == 1 and kc == FC // 2 - 1))
                    for b in bs:
                        t = moTp.tile([128, SE], F32, name="moT", tag="moT")
                        nc.vector.tensor_scalar_add(t[:], ps2[b][:],
                                                    f2b_sb[:, mc:mc + 1])
                        moT[b][mc] = t
                for b in bs:
                    for j, (o, pz) in enumerate(SQ):
                        x1res = xio.tile([pz, D], F32, name="x1r", tag="xin")
                        nc.sync.dma_start(x1res[:], x1_scr[b, o:o + pz, :])
                        ot = outnp.tile([pz, D], F32, name="outn", tag="outn")
                        for hf in range(2):
                            ps = pt.tile([pz, 512], F32, name="psO", tag="pp")
                            for dl in range(4):
                                dc = hf * 4 + dl
                                nc.tensor.transpose(
                                    ps[:, dl * 128:(dl + 1) * 128],
                                    moT[b][dc][:, o:o + pz], ident[:128, :128])
                            nc.vector.tensor_tensor(
                                out=ot[:, hf * 512:(hf + 1) * 512], in0=ps[:],
                                in1=x1res[:, hf * 512:(hf + 1) * 512],
                                op=ALU.add)
                        nc.sync.dma_start(out_d[b, o:o + pz, :], ot[:])
            esE.close()
            esDE.close()

    nc.compile()
    return nc


_NC = None


def _get_nc():
    global _NC
    if _NC is None:
        _NC = build()
    return _NC


def _prep_inputs(inputs):
    f = lambda a: np.ascontiguousarray(np.asarray(a, dtype=np.float32))
    x = f(inputs["hidden_states"])
    h = lambda a: np.ascontiguousarray(a.astype(np.float16))
    shared = {
        "qwT": h(f(inputs["q_w"]).T * SCALE),
        "kwT": h(f(inputs["k_w"]).T),
        "vwT": h(f(inputs["v_w"]).T),
        "owT": h(f(inputs["o_w"]).T),
        "f1wT": h(f(inputs["fc1_w"]).T),
        "f2wT": h(f(inputs["fc2_w"]).T),
        "qb": f(inputs["q_b"]) * SCALE,
        "kb": f(inputs["k_b"]),
        "ob": f(inputs["o_b"]) + f(inputs["o_w"]) @ f(inputs["v_b"]),
        "f1b": f(inputs["fc1_b"]),
        "f2b": f(inputs["fc2_b"]),
        "g1": f(inputs["ln1_g"]),
        "b1": f(inputs["ln1_b"]),
        "g2": f(inputs["ln2_g"]),
        "b2": f(inputs["ln2_b"]),
    }
    shared = {k: np.ascontiguousarray(v) for k, v in shared.items()}
    in_maps = []
    for c in range(N_CORES):
        m = dict(shared)
        m["x"] = np.ascontiguousarray(x[c * NB:(c + 1) * NB])
        in_maps.append(m)
    return in_maps


def run(inputs, trace=False):
    nc = _get_nc()
    in_maps = _prep_inputs(inputs)
    res = bass_utils.run_bass_kernel_spmd(
        nc, in_maps, core_ids=list(range(N_CORES)), trace=trace)
    out = np.concatenate([res.results[c]["out"] for c in range(N_CORES)],
                         axis=0)
    return out, res


def kernel(**inputs):
    out, _ = run(inputs, trace=False)
    return out


# revision 17
# speedup vs baseline: 1.8206x; 1.0215x over previous
"""ChineseCLIPVisionLayer on 8 trn2 NeuronCores.

Sharding: pure data-parallel over batch (B=32 -> 4 per core), zero
collectives. Weights are host-transposed and replicated to every core.

Per-core pipeline (all activations that feed matmuls live in transposed
layout [D, S] so the contraction dim sits on SBUF partitions):
  LN1 (natural) -> PE-transpose -> h^T
  q^T,k^T (transposed out), v (natural out)
  per head: scores = q_h^T.T @ k_h^T -> softmax (no max-sub; scores ~ +-3)
            probs -> PE-transpose -> probs^T
            attn^T = v_h.T @ probs^T   (+v_b via bias: softmax rows sum to 1)
  out_proj -> attn_out^T -> PE-transpose back + residual -> x1 (natural)
  LN2 -> h2^T ; MLP in 2 batch-groups (weights streamed once per group)
  quick-gelu == Gelu_apprx_sigmoid on ACT; fc2 out -> transpose + residual.
Matmuls run in float32r (full PE rate at N>=256; fp32 would be 4x slower).
Pools are stage-scoped (manually closed ExitStacks) to fit SBUF.
"""

from contextlib import ExitStack

import numpy as np

import concourse.bass as bass
import concourse.mybir as mybir
import concourse.tile as tile
from concourse import bacc, bass_utils
from concourse.masks import make_identity

N_CORES = 8
B, S, D = 32, 257, 1024
H, HD = 16, 64
FF = 4096
EPS = 1e-5
SCALE = HD ** -0.5
NB = B // N_CORES  # batch elems per core

F32 = mybir.dt.float32
F32R = mybir.dt.float32r
F16 = mybir.dt.float16
AF = mybir.ActivationFunctionType
ALU = mybir.AluOpType

# sequence chunks (partition-dim tiling of S=257)
SQ = [(0, 128), (128, 128), (256, 1)]
SE = 258  # free-dim padded length: fp32r matmul needs even moving/dst size
DC = D // 128   # 8 chunks of the model dim
FC = FF // 128  # 32 chunks of the ff dim


def r(ap):
    return ap.bitcast(F32R)


def build():
    nc = bacc.Bacc("TRN2", target_bir_lowering=False, debug=False,
                   num_devices=N_CORES)

    def din(name, shape, dt=F32):
        return nc.dram_tensor(name, shape, dt, kind="ExternalInput").ap()

    x_d = din("x", [NB, S, D])
    qwT_d = din("qwT", [D, D], F16)
    kwT_d = din("kwT", [D, D], F16)
    vwT_d = din("vwT", [D, D], F16)
    owT_d = din("owT", [D, D], F16)
    f1wT_d = din("f1wT", [D, FF], F16)
    f2wT_d = din("f2wT", [FF, D], F16)
    qb_d = din("qb", [D])
    kb_d = din("kb", [D])
    ob_d = din("ob", [D])
    f1b_d = din("f1b", [FF])
    f2b_d = din("f2b", [D])
    g1_d = din("g1", [D])
    b1_d = din("b1", [D])
    g2_d = din("g2", [D])
    b2_d = din("b2", [D])
    out_d = nc.dram_tensor("out", [NB, S, D], F32, kind="ExternalOutput").ap()

    with tile.TileContext(nc) as tc:
        with ExitStack() as es:
            P = lambda name, bufs, **kw: es.enter_context(
                tc.tile_pool(name=name, bufs=bufs, **kw))
            const = P("const", 1)
            biasp = P("bias", 1)
            xio = P("xio", 2)
            stat = P("stat", 8)
            pp = P("pp", 8, space="PSUM")
            pt = pp
            dramp = P("dram", 1, space="DRAM")

            ident = const.tile([128, 128], F32)
            make_identity(nc, ident)
            ident16 = const.tile([128, 128], F16)
            make_identity(nc, ident16)
            ones16 = const.tile([128, 1], F16)
            nc.vector.memset(ones16[:], 1.0)
            epsc = const.tile([128, 1], F32)
            nc.vector.memset(epsc[:], EPS)

            def load_bias(dram, n):
                t = biasp.tile([128, n // 128], F32, name=f"bias_{dram.name}")
                nc.sync.dma_start(t[:], dram.rearrange("(c p) -> p c", p=128))
                return t

            qb_sb = load_bias(qb_d, D)
            kb_sb = load_bias(kb_d, D)
            ob_sb = load_bias(ob_d, D)
            f1b_sb = load_bias(f1b_d, FF)
            f2b_sb = load_bias(f2b_d, D)
            g1_sb = load_bias(g1_d, D)
            b1_sb = load_bias(b1_d, D)
            g2_sb = load_bias(g2_d, D)
            b2_sb = load_bias(b2_d, D)

            x1_scr = dramp.tile([NB, S, D], F32)

            def layer_norm(src_tiles, hpool):
                """src_tiles: 3 natural tiles [(pz, D)]; returns normalized
                (x-mu)*rstd tiles (gamma/beta applied at transpose evict)."""
                out_tiles = []
                for j, (o, pz) in enumerate(SQ):
                    xt = src_tiles[j]
                    st = stat.tile([pz, 2, 6], F32, name="st", tag="st")
                    nc.vector.bn_stats(st[:, 0, :], xt[:, 0:512])
                    nc.vector.bn_stats(st[:, 1, :], xt[:, 512:1024])
                    mv = stat.tile([pz, 2], F32, name="mv", tag="mv")
                    nc.vector.bn_aggr(mv[:], st[:])
                    rstd = stat.tile([pz, 1], F32, name="rstd", tag="rstd")
                    nc.scalar.activation(rstd[:], mv[:, 1:2], AF.Sqrt,
                                         bias=epsc[:pz, :])
                    nc.vector.reciprocal(rstd[:], rstd[:])
                    ht = hpool.tile([pz, D], F32, name="hn", tag="hn")
                    nc.vector.tensor_scalar(
                        out=ht[:], in0=xt[:], scalar1=mv[:, 0:1],
                        scalar2=rstd[:], op0=ALU.subtract, op1=ALU.mult)
                    out_tiles.append(ht)
                return out_tiles

            def transpose_to_T(nat_tiles, dst_pool, g_sb, bt_sb, tag):
                """nat tiles [(pz, D)] -> 8 tiles [128, S] of the transpose,
                evicted with per-partition scale g and bias bt."""
                outs = []
                for dc in range(DC):
                    ps = pt.tile([128, SE], F32, name="psT", tag="pp")
                    for j, (o, pz) in enumerate(SQ):
                        nc.tensor.transpose(
                            ps[:, o:o + pz],
                            nat_tiles[j][:, dc * 128:(dc + 1) * 128],
                            ident[:pz, :pz])
                    t = dst_pool.tile([128, SE], F16, name=f"{tag}", tag=tag)
                    nc.vector.tensor_scalar(
                        out=t[:], in0=ps[:], scalar1=g_sb[:, dc:dc + 1],
                        scalar2=bt_sb[:, dc:dc + 1], op0=ALU.mult, op1=ALU.add)
                    outs.append(t)
                return outs

            def project_T(wT_dram, rhs_per_b, bias_sb, dst_pool, tag, wpool,
                          func=AF.Identity, odt=F16):
                """y^T = wT.T @ rhs (+bias) for every batch elem.
                Streams wT in two m-halves of [128, DC, 512] to bound SBUF.
                Returns outs[b][mc] tiles [128, S]."""
                src = wT_dram.rearrange("(kc p) m -> p kc m", p=128)
                outs = [[None] * DC for _ in range(NB)]
                for half in range(2):
                    wt = wpool.tile([128, DC, 512], F16, name="pw", tag="pw")
                    nc.sync.dma_start(
                        wt[:], src[:, :, half * 512:(half + 1) * 512])
                    for b in range(NB):
                        for ml in range(4):
                            mc = half * 4 + ml
                            ps = pp.tile([128, SE], F32, name="psP", tag="pp")
                            for kc in range(DC):
                                nc.tensor.matmul(
                                    ps[:],
                                    wt[:, kc, ml * 128:(ml + 1) * 128],
                                    rhs_per_b[b][kc][:],
                                    start=(kc == 0), stop=(kc == DC - 1))
                            t = dst_pool.tile([128, SE], odt, name=tag,
                                              tag=tag)
                            if func is AF.Identity:
                                nc.vector.tensor_scalar_add(
                                    t[:], ps[:], bias_sb[:, mc:mc + 1])
                            else:
                                nc.scalar.activation(t[:], ps[:], func,
                                                     bias=bias_sb[:, mc:mc + 1])
                            outs[b][mc] = t
                return outs

            # ---------- stage A: load x, LN1, h^T ----------
            esA_HT = ExitStack()
            HTp = esA_HT.enter_context(
                tc.tile_pool(name="HT", bufs=NB * DC, side="right"))
            esA = ExitStack()
            hnat = esA.enter_context(
                tc.tile_pool(name="hnat", bufs=3, side="right"))
            HT = []
            for b in range(NB):
                xts = []
                for j, (o, pz) in enumerate(SQ):
                    xt = xio.tile([pz, D], F32, name="xin", tag="xin")
                    nc.sync.dma_start(xt[:], x_d[b, o:o + pz, :])
                    xts.append(xt)
                hts = layer_norm(xts, hnat)
                HT.append(transpose_to_T(hts, HTp, g1_sb, b1_sb, "HT"))

            # ---------- stage B: QKV ----------
            esA.close()  # hnat dead
            esBC = ExitStack()
            qTp = esBC.enter_context(tc.tile_pool(name="qT", bufs=NB * DC))
            kTp = esBC.enter_context(tc.tile_pool(name="kT", bufs=NB * DC))
            vp = esBC.enter_context(tc.tile_pool(name="vna", bufs=NB * 3))
            esB = ExitStack()
            pwB = esB.enter_context(tc.tile_pool(name="pwB", bufs=3))

            qT = project_T(qwT_d, HT, qb_sb, qTp, "qT", pwB)
            kT = project_T(kwT_d, HT, kb_sb, kTp, "kT", pwB)

            # v in natural layout [s, D] (no bias: folded into attn eviction)
            vna = []
            v_src = vwT_d.rearrange("(kc p) m -> p kc m", p=128)
            for half in range(2):
                wt = pwB.tile([128, DC, 512], F16, name="pw", tag="pw")
                nc.sync.dma_start(
                    wt[:], v_src[:, :, half * 512:(half + 1) * 512])
                for b in range(NB):
                    if half == 0:
                        vna.append([vp.tile([pz, D], F16, name="vna",
                                            tag="vna") for (o, pz) in SQ])
                    for j, (o, pz) in enumerate(SQ):
                        ps = pp.tile([128, 512], F32, name="psV", tag="pp")
                        for kc in range(DC):
                            nc.tensor.matmul(
                                ps[:pz, :],
                                HT[b][kc][:, o:o + pz],
                                wt[:, kc, :],
                                start=(kc == 0), stop=(kc == DC - 1))
                        nc.vector.tensor_copy(
                            vna[b][j][:, half * 512:(half + 1) * 512],
                            ps[:pz, :])
            esB.close()    # qkv weights dead
            esA_HT.close()  # HT dead

            # ---------- stage C: attention ----------
            # scoresT = k_h @ q_h^T directly (no probs transpose); softmax
            # denominator via ones-matmul column sums; normalization fused
            # into the DVE eviction of attn^T; v_b folded into o_b on host.
            esC = ExitStack()
            probsTp = esC.enter_context(tc.tile_pool(name="probsT", bufs=8))
            rcp = esC.enter_context(tc.tile_pool(name="rcp", bufs=3))
            esCD = ExitStack()
            attnTp = esCD.enter_context(
                tc.tile_pool(name="attnT", bufs=NB * DC, side="right"))
            attnT = []
            for b in range(NB):
                attnT.append([None] * DC)
                # pass 1: scoresT, exp; per-head column sums land at psum
                # partitions 0/32/64/96 (PE tile_position) so one 128-lane
                # reciprocal serves 4 heads
                for h in range(H):
                    dc, po = h // 2, (h % 2) * 64
                    pTs = []
                    csum = pp.tile([1, SE], F32, name="psCS", tag="pp")
                    for sj, (so, spz) in enumerate(SQ):
                        scT = pp.tile([128, SE], F32, name="psS", tag="pp")
                        nc.tensor.matmul(
                            scT[:spz, :],
                            kT[b][dc][po:po + 64, so:so + spz],
                            qT[b][dc][po:po + 64, :],
                            start=True, stop=True)
                        pT = probsTp.tile([spz, SE], F16, name="pT", tag="pT")
                        nc.scalar.activation(pT[:], scT[:spz, :], AF.Exp)
                        pTs.append(pT)
                        nc.tensor.matmul(
                            csum[:], ones16[:spz, :], pTs[sj][:],
                            start=(sj == 0), stop=(sj == 2))
                    rc = rcp.tile([1, SE], F32, name="rc", tag="rc")
                    nc.vector.reciprocal_approx_fast(rc[:], csum[:])
                    rcb = rcp.tile([64, SE], F32, name="rcb", tag="rcb")
                    nc.gpsimd.partition_broadcast(rcb[:], rc[:])
                    at = pp.tile([64, SE], F32, name="psA", tag="pp")
                    for sj, (so, spz) in enumerate(SQ):
                        nc.tensor.matmul(
                            at[:], vna[b][sj][:, h * 64:(h + 1) * 64],
                            pTs[sj][:],
                            start=(sj == 0), stop=(sj == 2))
                    if po == 0:
                        attnT[b][dc] = attnTp.tile([128, SE], F16,
                                                   name="atT", tag="atT")
                    nc.vector.tensor_tensor(
                        out=attnT[b][dc][po:po + 64, :], in0=at[:],
                        in1=rcb[:], op=ALU.mult)
            esC.close()   # probsT, rcp dead
            esBC.close()  # qT, kT, vna dead

            # ---------- stage D: out_proj, residual, LN2, h2^T ----------
            esD2 = ExitStack()
            aoTp = esD2.enter_context(tc.tile_pool(name="aoT", bufs=NB * DC))
            esD = ExitStack()
            pwD = esD.enter_context(tc.tile_pool(name="pwD", bufs=3))
            aoT = project_T(owT_d, attnT, ob_sb, aoTp, "aoT", pwD, odt=F32)
            esD.close()   # ow weights dead
            esCD.close()  # attnT dead

            esDE = ExitStack()
            H2Tp = esDE.enter_context(
                tc.tile_pool(name="H2T", bufs=NB * DC, side="right"))
            esD3 = ExitStack()
            x1p = esD3.enter_context(
                tc.tile_pool(name="x1", bufs=3, side="right"))
            h2natp = esD3.enter_context(
                tc.tile_pool(name="h2nat", bufs=3, side="right"))
            H2T = []
            for b in range(NB):
                x1ts = []
                for j, (o, pz) in enumerate(SQ):
                    xres = xio.tile([pz, D], F32, name="xres", tag="xin")
                    nc.sync.dma_start(xres[:], x_d[b, o:o + pz, :])
                    x1t = x1p.tile([pz, D], F32, name="x1", tag="x1")
                    for hf in range(2):
                        ps = pt.tile([pz, 512], F32, name="psN", tag="pp")
                        for dl in range(4):
                            dc = hf * 4 + dl
                            nc.tensor.transpose(
                                ps[:, dl * 128:(dl + 1) * 128],
                                aoT[b][dc][:, o:o + pz], ident[:128, :128])
                        nc.vector.tensor_tensor(
                            out=x1t[:, hf * 512:(hf + 1) * 512], in0=ps[:],
                            in1=xres[:, hf * 512:(hf + 1) * 512], op=ALU.add)
                    nc.sync.dma_start(x1_scr[b, o:o + pz, :], x1t[:])
                    x1ts.append(x1t)
                h2ts = layer_norm(x1ts, h2natp)
                H2T.append(transpose_to_T(h2ts, H2Tp, g2_sb, b2_sb, "H2T"))
            esD2.close()  # aoT dead
            esD3.close()  # x1, h2nat dead

            # ---------- stage E: MLP in 2 batch groups ----------
            esE = ExitStack()
            w1p = esE.enter_context(tc.tile_pool(name="w1", bufs=2))
            w2p = esE.enter_context(tc.tile_pool(name="w2", bufs=3))
            h1Tp = esE.enter_context(tc.tile_pool(name="h1T", bufs=2 * FC))
            moTp = esE.enter_context(tc.tile_pool(name="moT", bufs=2 * DC))
            outnp = esE.enter_context(tc.tile_pool(name="outn", bufs=2))
            f1_src = f1wT_d.rearrange("(kc p) m -> p kc m", p=128)
            f2_src = f2wT_d.rearrange("(kc p) m -> p kc m", p=128)
            for grp in range(2):
                bs = [grp * 2, grp * 2 + 1]
                h1T = {b: [None] * FC for b in bs}
                for mc in range(FC):
                    w1t = w1p.tile([128, DC, 128], F16, name="w1", tag="w1")
                    nc.sync.dma_start(
                        w1t[:], f1_src[:, :, mc * 128:(mc + 1) * 128])
                    for b in bs:
                        ps = pp.tile([128, SE], F32, name="psF1", tag="pp")
                        for kc in range(DC):
                            nc.tensor.matmul(
                                ps[:], w1t[:, kc, :], H2T[b][kc][:],
                                start=(kc == 0), stop=(kc == DC - 1))
                        t = h1Tp.tile([128, SE], F16, name="h1T",
                                      tag="h1T")
                        nc.scalar.activation(t[:], ps[:],
                                             AF.Gelu_apprx_sigmoid,
                                             bias=f1b_sb[:, mc:mc + 1])
                        h1T[b][mc] = t
                moT = {b: [None] * DC for b in bs}
                for mc in range(DC):
                    for kh in range(2):
                        w2t = w2p.tile([128, FC // 2, 128], F16, name="w2",
                                       tag="w2")
                        nc.sync.dma_start(
                            w2t[:], f2_src[:, kh * 16:(kh + 1) * 16,
                                           mc * 128:(mc + 1) * 128])
                        if kh == 0:
                            ps2 = {b: pp.tile([128, SE], F32, name="psF2",
                                              tag="pp") for b in bs}
                        for b in bs:
                            for kc in range(FC // 2):
                                nc.tensor.matmul(
                                    ps2[b][:], w2t[:, kc, :],
                                    h1T[b][kh * 16 + kc][:],
                                    start=(kh == 0 and kc == 0),
                                    stop=(kh == 1 and kc == FC // 2 - 1))
                    for b in bs:
                        t = moTp.tile([128, SE], F32, name="moT", tag="moT")
                        nc.vector.tensor_scalar_add(t[:], ps2[b][:],
                                                    f2b_sb[:, mc:mc + 1])
                        moT[b][mc] = t
                for b in bs:
                    for j, (o, pz) in enumerate(SQ):
                        x1res = xio.tile([pz, D], F32, name="x1r", tag="xin")
                        nc.sync.dma_start(x1res[:], x1_scr[b, o:o + pz, :])
                        ot = outnp.tile([pz, D], F32, name="outn", tag="outn")
                        for hf in range(2):
                            ps = pt.tile([pz, 512], F32, name="psO", tag="pp")
                            for dl in range(4):
                                dc = hf * 4 + dl
                                nc.tensor.transpose(
                                    ps[:, dl * 128:(dl + 1) * 128],
                                    moT[b][dc][:, o:o + pz], ident[:128, :128])
                            nc.vector.tensor_tensor(
                                out=ot[:, hf * 512:(hf + 1) * 512], in0=ps[:],
                                in1=x1res[:, hf * 512:(hf + 1) * 512],
                                op=ALU.add)
                        nc.sync.dma_start(out_d[b, o:o + pz, :], ot[:])
            esE.close()
            esDE.close()

    nc.compile()
    return nc


_NC = None


def _get_nc():
    global _NC
    if _NC is None:
        _NC = build()
    return _NC


def _prep_inputs(inputs):
    f = lambda a: np.ascontiguousarray(np.asarray(a, dtype=np.float32))
    x = f(inputs["hidden_states"])
    h = lambda a: np.ascontiguousarray(a.astype(np.float16))
    shared = {
        "qwT": h(f(inputs["q_w"]).T * SCALE),
        "kwT": h(f(inputs["k_w"]).T),
        "vwT": h(f(inputs["v_w"]).T),
        "owT": h(f(inputs["o_w"]).T),
        "f1wT": h(f(inputs["fc1_w"]).T),
        "f2wT": h(f(inputs["fc2_w"]).T),
        "qb": f(inputs["q_b"]) * SCALE,
        "kb": f(inputs["k_b"]),
        "ob": f(inputs["o_b"]) + f(inputs["o_w"]) @ f(inputs["v_b"]),
        "f1b": f(inputs["fc1_b"]),
        "f2b": f(inputs["fc2_b"]),
        "g1": f(inputs["ln1_g"]),
        "b1": f(inputs["ln1_b"]),
        "g2": f(inputs["ln2_g"]),
        "b2": f(inputs["ln2_b"]),
    }
    shared = {k: np.ascontiguousarray(v) for k, v in shared.items()}
    in_maps = []
    for c in range(N_CORES):
        m = dict(shared)
        m["x"] = np.ascontiguousarray(x[c * NB:(c + 1) * NB])
        in_maps.append(m)
    return in_maps


def run(inputs, trace=False):
    nc = _get_nc()
    in_maps = _prep_inputs(inputs)
    res = bass_utils.run_bass_kernel_spmd(
        nc, in_maps, core_ids=list(range(N_CORES)), trace=trace)
    out = np.concatenate([res.results[c]["out"] for c in range(N_CORES)],
                         axis=0)
    return out, res


def kernel(**inputs):
    out, _ = run(inputs, trace=False)
    return out
